# revision 1
# baseline (speedup 1.0000x reference)
"""Trainium2 Bass kernel for nn_Join: out = concat(unary[idx1], unary[idx2], binary).

Strategy (data-parallel over edges, 8 cores):
  - 1M edges sharded 125000/core, padded to a multiple of 128.
  - unary table (51.2MB fp32) replicated per core; gathers are local
    HW indirect DMAs. The HW DGE supports exactly one index per SBUF
    partition per call, so rows are tiled p-outer: row = p*ncols + t.
    Gather block t uses offset column idx_sb[:, t] and lands 128 rows
    (512B each) in the out tile's column block t.
  - A supertile of S blocks shares one binary load, one DVE copy into
    the 256:320 column slot, and one large contiguous store.
  - With row = p*ncols + t, all DRAM APs are plain reshapes of the
    natural row order: no host-side permutation of any tensor.
"""

import numpy as np
from contextlib import ExitStack

import concourse.bass as bass
import concourse.bacc as bacc
import concourse.tile as tile
import concourse.mybir as mybir
from concourse.bass_utils import run_bass_kernel_spmd

N_CORES = 8
U_NODES, U_DIM = 100000, 128
B_DIM = 64
OUT_DIM = 2 * U_DIM + B_DIM  # 320
P = 128
SUPER = 16  # gather blocks (columns) per supertile
B_EDGES = 1000000


def _build_nc(ncols: int, out_bufs: int = 4, b_bufs: int = 3, super_s: int = SUPER):
    ne_pad = ncols * P
    nc = bacc.Bacc(
        "TRN2",
        target_bir_lowering=False,
        debug=False,
        enable_asserts=False,
        num_devices=N_CORES,
    )
    unary = nc.dram_tensor(
        "unary", [U_NODES, U_DIM], mybir.dt.float32, kind="ExternalInput"
    ).ap()
    binary = nc.dram_tensor(
        "binary", [ne_pad, B_DIM], mybir.dt.float32, kind="ExternalInput"
    ).ap()
    idx1 = nc.dram_tensor("idx1", [P, ncols], mybir.dt.int32, kind="ExternalInput").ap()
    idx2 = nc.dram_tensor("idx2", [P, ncols], mybir.dt.int32, kind="ExternalInput").ap()
    out = nc.dram_tensor(
        "out", [ne_pad, OUT_DIM], mybir.dt.float32, kind="ExternalOutput"
    ).ap()

    bin_v = binary.rearrange("(p n) c -> p n c", p=P)  # [128, ncols, 64]
    out_v = out.rearrange("(p n) c -> p n c", p=P)  # [128, ncols, 320]

    with tile.TileContext(nc) as tc, ExitStack() as ctx:
        idx_pool = ctx.enter_context(tc.tile_pool(name="idx", bufs=1))
        ot_pool = ctx.enter_context(tc.tile_pool(name="ot", bufs=out_bufs))
        bt_pool = ctx.enter_context(tc.tile_pool(name="bt", bufs=b_bufs))

        idx1_sb = idx_pool.tile([P, ncols], mybir.dt.int32, tag="idx1")
        idx2_sb = idx_pool.tile([P, ncols], mybir.dt.int32, tag="idx2")
        nc.sync.dma_start(idx1_sb[:], idx1[:, :])
        nc.sync.dma_start(idx2_sb[:], idx2[:, :])

        c0 = 0
        while c0 < ncols:
            S = min(super_s, ncols - c0)
            ot = ot_pool.tile([P, S * OUT_DIM], mybir.dt.float32, tag="ot")
            ov = ot[:].rearrange("p (s c) -> p s c", c=OUT_DIM)
            for s in range(S):
                nc.gpsimd.indirect_dma_start(
                    out=ov[:, s, 0:U_DIM],
                    out_offset=None,
                    in_=unary[:, :],
                    in_offset=bass.IndirectOffsetOnAxis(
                        ap=idx1_sb[:, c0 + s : c0 + s + 1], axis=0
                    ),
                )
                nc.gpsimd.indirect_dma_start(
                    out=ov[:, s, U_DIM : 2 * U_DIM],
                    out_offset=None,
                    in_=unary[:, :],
                    in_offset=bass.IndirectOffsetOnAxis(
                        ap=idx2_sb[:, c0 + s : c0 + s + 1], axis=0
                    ),
                )
            nc.sync.dma_start(ov[:, :, 2 * U_DIM : OUT_DIM], bin_v[:, c0 : c0 + S, :])
            nc.sync.dma_start(out_v[:, c0 : c0 + S, :], ot[:])
            c0 += S

    nc.compile()
    return nc


_NC_CACHE: dict = {}


def _get_nc(ncols: int):
    if ncols not in _NC_CACHE:
        _NC_CACHE[ncols] = _build_nc(ncols)
    return _NC_CACHE[ncols]


def kernel(unary, binary, index1, index2):
    unary = np.ascontiguousarray(np.asarray(unary, dtype=np.float32))
    binary = np.ascontiguousarray(np.asarray(binary, dtype=np.float32))
    index1 = np.asarray(index1).astype(np.int32).ravel()
    index2 = np.asarray(index2).astype(np.int32).ravel()

    ne_total = binary.shape[0]
    per_core = -(-ne_total // N_CORES)
    ncols = -(-per_core // P)
    ne_pad = ncols * P
    nc = _get_nc(ncols)

    in_maps = []
    counts = []
    for c in range(N_CORES):
        lo = c * per_core
        hi = min(lo + per_core, ne_total)
        n = hi - lo
        counts.append(n)
        b = np.zeros((ne_pad, B_DIM), dtype=np.float32)
        b[:n] = binary[lo:hi]
        i1 = np.zeros(ne_pad, dtype=np.int32)
        i1[:n] = index1[lo:hi]
        i2 = np.zeros(ne_pad, dtype=np.int32)
        i2[:n] = index2[lo:hi]
        in_maps.append(
            {
                "unary": unary,
                "binary": b,
                "idx1": np.ascontiguousarray(i1.reshape(P, ncols)),
                "idx2": np.ascontiguousarray(i2.reshape(P, ncols)),
            }
        )

    res = run_bass_kernel_spmd(nc, in_maps, core_ids=list(range(N_CORES)))
    out = np.empty((ne_total, OUT_DIM), dtype=np.float32)
    row = 0
    for c in range(N_CORES):
        out[row : row + counts[c]] = res.results[c]["out"][: counts[c]]
        row += counts[c]
    return out

